# revision 2
# baseline (speedup 1.0000x reference)
"""AdaptiveTokenMixer Trainium2 kernel (8 NeuronCores, pure data parallel).

Per-core algorithm (one batch element per core):
  1. alpha stage (DVE/ACT): sliding-window loads of delta_times/valid_mask,
     masked temporal-decay softmax over K=8 offsets, blended with host-
     precomputed (b/(1-b))*softmax(w) (scale-invariant under the final
     renormalization), masked + renormalized -> alpha [N, 8] bf16.
  2. W stage (DMA only): alpha is written to a DRAM scratch with a SKEWED
     access pattern, forming a banded matrix W^T[m, k] = alpha[n0+m, k-m]
     (m-major, 128x128 per 120-position block), then transpose-loaded back
     (bf16 XBAR DMA transpose) as W[k, m] in SBUF.
  3. Mix stage (PE): out[m, :] = sum_k W[k, m] * x[n0+k, :] -- one 128x128 @
     128x256 matmul per block accumulating the K-tap mixing exactly.
  4. Evict PSUM -> SBUF bf16 (DVE/ACT alternating), DMA to DRAM.

Self-contained: hardcodes shapes for B=8, N=4096, d=256, K=8.
"""
import numpy as np
import ml_dtypes

import concourse.bass as bass
import concourse.bacc as bacc
import concourse.mybir as mybir
from concourse import tile
from concourse.bass_utils import run_bass_kernel_spmd

B, N, D, K = 8, 4096, 256, 8
BLK = 120                      # output positions per block
NB = (N + BLK - 1) // BLK      # 35 blocks -> covers 4200 positions
NOUT = NB * BLK                # 4200 rows in padded device output
NPAD = 4224                    # padded input length (>= 34*120 + 128 + 8)
KW = 128                       # k-window (contraction) per block
WBLK = KW * KW                 # W scratch elements per block
F = K * NB                     # alpha free size (p-major, b-minor)
BIG = 1024.0

_CACHE = {}


def _build():
    nc = bacc.Bacc("TRN2", target_bir_lowering=False, debug=False,
                   num_devices=B)
    f32 = mybir.dt.float32
    bf16 = mybir.dt.bfloat16

    x_t = nc.dram_tensor("x", [NPAD, D], bf16, kind="ExternalInput")
    dt_t = nc.dram_tensor("dt", [NPAD], f32, kind="ExternalInput")
    vf_t = nc.dram_tensor("vf", [NPAD], f32, kind="ExternalInput")
    bwsm_t = nc.dram_tensor("bwsm", [128, K], f32, kind="ExternalInput")
    out_t = nc.dram_tensor("out", [NOUT, D], bf16, kind="ExternalOutput")
    wdram = nc.dram_tensor("wscratch", [NB * WBLK], bf16, kind="Internal")

    def pb(t):  # [128,(p,b)] view -> [128, b, p] (p innermost, for reduces)
        return bass.AP(t.tensor, t.offset, [t.ap[0], [1, NB], [NB, K]])

    def strip(t, col):  # p-strip [128, NB]
        return t[:, col * NB: (col + 1) * NB]

    def exp_nb(a):  # [128, NB] AP -> [128, (K-rep), NB]
        return bass.AP(a.tensor, a.offset, [a.ap[0], [0, K], [1, NB]])

    def exp_k(a):  # [128, K] AP -> [128, K, (NB-rep)]
        return bass.AP(a.tensor, a.offset, [a.ap[0], [1, K], [0, NB]])

    with tile.TileContext(nc) as tc:
        with tc.tile_pool(name="alph", bufs=1) as apool, \
             tc.tile_pool(name="mix", bufs=6) as mpool, \
             tc.tile_pool(name="psum", bufs=6, space="PSUM") as ppool:

            # --- zero-fill W scratch ---
            ztile = apool.tile([128, KW], bf16)
            nc.vector.memset(ztile[:], 0.0)
            for b in range(NB):
                nc.sync.dma_start(
                    bass.AP(wdram, b * WBLK, [[KW, 128], [1, KW]]), ztile[:])

            # --- alpha stage ---
            dtw = apool.tile([128, F], f32)
            vw = apool.tile([128, F], f32)
            for p in range(K):
                nc.sync.dma_start(
                    strip(dtw, p), bass.AP(dt_t, p, [[1, 128], [BLK, NB]]))
                nc.sync.dma_start(
                    strip(vw, p), bass.AP(vf_t, p, [[1, 128], [BLK, NB]]))
            bwsm = apool.tile([128, K], f32)
            nc.sync.dma_start(bwsm[:], bwsm_t.ap())

            t1 = apool.tile([128, F], f32)
            nc.vector.tensor_scalar(t1[:], dtw[:], -1.0, BIG,
                                    mybir.AluOpType.mult, mybir.AluOpType.add)
            cv = apool.tile([128, F], f32)
            nc.vector.tensor_tensor(cv[:], vw[:], exp_nb(strip(vw, 0)),
                                    mybir.AluOpType.mult)
            lg = apool.tile([128, F], f32)
            nc.vector.tensor_tensor(lg[:], t1[:], cv[:], mybir.AluOpType.mult)
            mx = apool.tile([128, NB], f32)
            nc.vector.tensor_reduce(mx[:], pb(lg), mybir.AxisListType.X,
                                    mybir.AluOpType.max)
            ei = apool.tile([128, F], f32)
            nc.vector.tensor_tensor(ei[:], lg[:], exp_nb(mx[:, :]),
                                    mybir.AluOpType.subtract)
            e = apool.tile([128, F], f32)
            nc.scalar.activation(e[:], ei[:], mybir.ActivationFunctionType.Exp)
            s = apool.tile([128, NB], f32)
            nc.vector.tensor_reduce(s[:], pb(e), mybir.AxisListType.X,
                                    mybir.AluOpType.add)
            rcp = apool.tile([128, NB], f32)
            nc.vector.reciprocal(rcp[:], s[:])
            th = apool.tile([128, F], f32)
            nc.vector.tensor_tensor(th[:], e[:], exp_nb(rcp[:, :]),
                                    mybir.AluOpType.mult)
            au = apool.tile([128, F], f32)
            nc.vector.tensor_tensor(au[:], th[:], exp_k(bwsm[:, :]),
                                    mybir.AluOpType.add)
            nc.vector.tensor_tensor(au[:], au[:], cv[:], mybir.AluOpType.mult)
            sa = apool.tile([128, NB], f32)
            nc.vector.tensor_reduce(sa[:], pb(au), mybir.AxisListType.X,
                                    mybir.AluOpType.add)
            nc.vector.tensor_scalar(sa[:], sa[:], 1e-8, None,
                                    mybir.AluOpType.max)
            r = apool.tile([128, NB], f32)
            nc.vector.reciprocal(r[:], sa[:])
            nc.vector.tensor_tensor(r[:], r[:], strip(vw, 0),
                                    mybir.AluOpType.mult)
            af = apool.tile([128, F], bf16)
            nc.vector.tensor_tensor(af[:], au[:], exp_nb(r[:, :]),
                                    mybir.AluOpType.mult)

            # --- skewed W write: W^T[b][m, m+p] = alpha[b*120+m, p] ---
            for b in range(NB):
                nc.sync.dma_start(
                    bass.AP(wdram, b * WBLK, [[KW + 1, BLK], [1, K]]),
                    bass.AP(af.tensor, af.offset + b,
                            [af.ap[0], [NB, K]])[:BLK, :])

            # --- mix stage ---
            for b in range(NB):
                wt = mpool.tile([KW, KW], bf16, tag="w")
                nc.scalar.dma_start_transpose(
                    wt[:], bass.AP(wdram, b * WBLK, [[KW, KW], [1, KW]]))
                xt = mpool.tile([KW, D], bf16, tag="x")
                nc.sync.dma_start(xt[:], x_t.ap()[b * BLK: b * BLK + KW, :])
                pt = ppool.tile([KW, D], f32)
                nc.tensor.matmul(pt[:], wt[:], xt[:])
                ot = mpool.tile([KW, D], bf16, tag="o")
                if b % 2 == 0:
                    nc.scalar.copy(ot[:BLK, :], pt[:BLK, :])
                else:
                    nc.vector.tensor_copy(ot[:BLK, :], pt[:BLK, :])
                nc.sync.dma_start(out_t.ap()[b * BLK: b * BLK + BLK, :],
                                  ot[:BLK, :])
    nc.compile()
    return nc


def _get_nc():
    if "nc" not in _CACHE:
        _CACHE["nc"] = _build()
    return _CACHE["nc"]


def _make_in_maps(x, delta_times, valid_mask, w, beta):
    w64 = w.astype(np.float64)
    wsm = np.exp(w64 - w64.max())
    wsm /= wsm.sum()
    b = 1.0 / (1.0 + np.exp(-float(beta[0])))
    bwsm = np.tile((b / (1.0 - b) * wsm)[None, :], (128, 1)).astype(np.float32)

    in_maps = []
    for i in range(B):
        xp = np.zeros((NPAD, D), np.float32)
        xp[:N] = x[i]
        dtp = np.zeros(NPAD, np.float32)
        dtp[:N] = delta_times[i]
        vfp = np.zeros(NPAD, np.float32)
        vfp[:N] = valid_mask[i].astype(np.float32)
        in_maps.append({
            "x": xp.astype(ml_dtypes.bfloat16),
            "dt": dtp,
            "vf": vfp,
            "bwsm": bwsm,
        })
    return in_maps


def _execute(in_maps, trace=False, **kw):
    nc = _get_nc()
    return run_bass_kernel_spmd(nc, in_maps, core_ids=list(range(B)),
                                trace=trace, **kw)


def kernel(x, delta_times, valid_mask, w, beta):
    in_maps = _make_in_maps(x, delta_times, valid_mask, w, beta)
    kr = _execute(in_maps, trace=False)
    outs = [kr.results[i]["out"][:N].astype(np.float32) for i in range(B)]
    return np.stack(outs, axis=0)
